# revision 36
# baseline (speedup 1.0000x reference)
"""Distributed multi-head attention layer on 8 TRN2 NeuronCores.

Problem: B=2, S=2048, D=1024, H=16 heads, head_dim=64, fp32.

Sharding: sequence-parallel over query rows, with REDUNDANT per-core
K/V computation instead of collectives (a 4-rank AllGather costs
~15us + size/40GBps here — far more than the extra matmuls). The
4096 global rows split into 8 chunks of 512; core c owns query rows
[512c, 512c+512), all in batch b = c // 4. The host permutes each
core's batch so its own 512 rows come first (attention is
permutation-invariant over keys). Q^T and the output projection
cover only chunk 0; K^T/V cover all 4 chunks of the batch.

Schedule: prologue projects V/K of chunk 0 and Q^T slabs 0-1; then 4
"sweeps", one per key chunk. Q^T slabs 2-7 ride sweep-0 filler slots
and chunk-3 K slabs 3-7 defer into sweep 3, so the DMA-gated prologue
stays short and the filler-less last sweep is not Scalar-starved.
Weight DMAs stream in consumption order, each spread across all three
DMA queues, and the x ring holds two chunks so chunk r+1's load
overlaps sweep r-1. Sweep r runs attention for all 8 head
pairs over chunk r's 4 key tiles, with the V/K projections of chunk
r+1 interleaved between head pairs so the PE stays busy while the
Scalar engine computes the softmax Exps. Attended values accumulate
in PSUM within a sweep (4 contiguous matmuls — interrupting an
accumulation group costs ~2x on the resume) and in an SBUF bf16
accumulator across sweeps (DVE add).

Per (pair, key-tile): both heads' scores land in one [128,1024] PSUM
tile via two matmuls on PE row groups 0/64 (which execute
concurrently), one Exp activation covers both, and the ones-column of
the V tiles produces the softmax denominator inside the attended
matmul. Normalization (broadcast fast-reciprocal multiply) is emitted
inside sweep 3 right after each pair's last accumulate, so the 16
chains overlap sweep-3 compute and the hoisted output projection.

Empirically rejected variants (each re-measured at +55us): K/Q bias
adds on DVE instead of Scalar (DVE back-pressure stalls PSUM slot
release), mid-sweep x-chunk DMA dispatch moved off the Scalar queue,
and a DRAM AllGather for K/V (15us + size/40GBps makes remote K/V
arrive far too late; redundant compute wins).
"""

import sys

sys.path.insert(0, "/opt/trn_rl_repo")

import ml_dtypes
import numpy as np

import concourse.bass as bass
import concourse.tile as tile
from concourse import bacc, mybir
from concourse.bass_utils import run_bass_kernel_spmd

f32 = mybir.dt.float32
f32r = mybir.dt.float32r
bf16 = mybir.dt.bfloat16
ACT = mybir.ActivationFunctionType

B, S, D = 2, 2048, 1024
H, HD = 16, 64
NCORES = 8
R = (B * S) // NCORES  # 512 query rows per core
NK = D // 128  # 8 contraction tiles
NKT = S // 128  # 16 key tiles per batch
RCH = S // 512  # 4 row-chunks of the batch
NHP = H // 2  # 8 head pairs
SCALE = 1.0 / float(np.sqrt(HD))

_COMPILED = {}


def build_nc(use_f32r=True):
    nc = bacc.Bacc("TRN2", target_bir_lowering=False, debug=False, num_devices=8)

    xTb = nc.dram_tensor("xTb", [D, S], bf16, kind="ExternalInput")
    Wq = nc.dram_tensor("Wq", [D, D], bf16, kind="ExternalInput")
    Wk = nc.dram_tensor("Wk", [D, D], bf16, kind="ExternalInput")
    Wv = nc.dram_tensor("Wv", [D, D], bf16, kind="ExternalInput")
    Wo = nc.dram_tensor("Wo", [D, D], bf16, kind="ExternalInput")
    bq = nc.dram_tensor("bq", [D, 1], f32, kind="ExternalInput")
    bk = nc.dram_tensor("bk", [D, 1], f32, kind="ExternalInput")
    bv = nc.dram_tensor("bv", [D, 1], f32, kind="ExternalInput")
    bo = nc.dram_tensor("bo", [D, 1], f32, kind="ExternalInput")
    out = nc.dram_tensor("out", [R, D], f32, kind="ExternalOutput")

    def bcast_row(handle):
        # [D,1] bias -> [128, D] partition-broadcast DMA source
        return bass.AP(tensor=handle.ap().tensor, offset=0, ap=[[0, 128], [1, D]])

    with tile.TileContext(nc) as tc:
        with (
            # wk/wv resident through sweep 2; wq only for the prologue
            tc.tile_pool(name="wpool", bufs=24) as wpool,
            tc.tile_pool(name="wopool", bufs=8) as wopool,
            # streamed x^T tiles, 2-chunk ring: chunk r+2's DMA starts as
            # soon as chunk r's last reader retires
            tc.tile_pool(name="xbt", bufs=16) as xbt_pool,
            # K^T of the whole batch, bf16, feature-major: 8 x [128, 2048]
            tc.tile_pool(name="k2", bufs=8) as k2_pool,
            # ones-augmented V of the whole batch, bf16: 16 x [128, 16*65]
            tc.tile_pool(name="vaug", bufs=16) as vaug_pool,
            tc.tile_pool(name="qt_pool", bufs=8) as qt_pool,
            # cross-sweep attended accumulators, bf16: 16 x [65, 512]
            tc.tile_pool(name="aacc", bufs=16) as aacc_pool,
            tc.tile_pool(name="att_pool", bufs=8) as att_pool,
            tc.tile_pool(name="exp_pool", bufs=5) as exp_pool,
            tc.tile_pool(name="outp", bufs=2) as outp,
            tc.tile_pool(name="bias", bufs=1) as bias_pool,
            tc.tile_pool(name="small", bufs=2) as small,
            tc.tile_pool(name="ps_mm", bufs=2, space="PSUM") as ps_mm,
            tc.tile_pool(name="ps_sc", bufs=2, space="PSUM") as ps_sc,
            tc.tile_pool(name="ps_att", bufs=2, space="PSUM") as ps_att,
        ):
            dma_round = [nc.sync, nc.scalar, nc.gpsimd]

            def load_chunk(r):
                tiles = []
                for k in range(NK):
                    xb = xbt_pool.tile([128, 512], bf16, name=f"xb{r}_{k}", tag="xb")
                    dma_round[k % 3].dma_start(
                        out=xb,
                        in_=xTb.ap()[
                            128 * k : 128 * (k + 1), 512 * r : 512 * (r + 1)
                        ],
                    )
                    tiles.append(xb)
                return tiles

            # weights stream in consumption order, each spread across all
            # three DMA queues so the first projection units unblock early:
            # Wv (V units lead the prologue), chunk 0, Wk, Wq (sweep-0
            # fillers), Wo (output projection)
            def load_w(pool, tag, dram, k, dt=bf16):
                t = pool.tile([128, D], dt, name=f"{tag}{k}", tag="wo" if tag == "wo" else "w", bufs=8)
                dma_round[k % 3].dma_start(out=t, in_=dram.ap()[128 * k : 128 * (k + 1), :])
                return t

            # Wv/Wk load as column halves so the first projection units
            # need only the half they consume
            def load_half(tag, dram, k, h):
                t = wpool.tile(
                    [128, D // 2], bf16, name=f"{tag}{k}_{h}", tag="wh", bufs=32
                )
                dma_round[k % 3].dma_start(
                    out=t,
                    in_=dram.ap()[128 * k : 128 * (k + 1), 512 * h : 512 * (h + 1)],
                )
                return t

            wv_sb = [[load_half("wv", Wv, k, 0) for k in range(NK)], [None] * NK]
            x_chunk = [load_chunk(0)]
            wk_sb = [[load_half("wk", Wk, k, 0) for k in range(NK)], [None] * NK]
            for k in range(NK):
                wv_sb[1][k] = load_half("wv", Wv, k, 1)
            for k in range(NK):
                wk_sb[1][k] = load_half("wk", Wk, k, 1)
            x_chunk += [load_chunk(r) for r in range(1, RCH)]
            wq_sb = [load_w(wpool, "wq", Wq, k) for k in range(NK)]
            wo_sb = [load_w(wopool, "wo", Wo, k) for k in range(NK)]

            bk_sb = bias_pool.tile([128, NK], f32)
            nc.gpsimd.dma_start(
                out=bk_sb, in_=bk.ap().rearrange("(k p) one -> p k one", p=128)
            )
            bq_sb = bias_pool.tile([128, NK], f32)
            nc.gpsimd.dma_start(
                out=bq_sb, in_=bq.ap().rearrange("(k p) one -> p k one", p=128)
            )
            bv_bc = bias_pool.tile([128, D], f32, name="bv_bc", tag="bc")
            nc.gpsimd.dma_start(out=bv_bc, in_=bcast_row(bv))

            k2_sb = [
                k2_pool.tile([128, S], bf16, name=f"k2_{m}", tag="k2")
                for m in range(NK)
            ]
            ones_dram = nc.inline_tensor(
                np.ones((1, H), ml_dtypes.bfloat16), name="ones16"
            )
            ones_sb = bias_pool.tile([128, H], bf16, name="ones_sb", tag="ones")
            nc.gpsimd.dma_start(
                out=ones_sb,
                in_=bass.AP(tensor=ones_dram, offset=0, ap=[[0, 128], [1, H]]),
            )
            vaug_sb = []
            for kt in range(NKT):
                va = vaug_pool.tile([128, H * 65], bf16, name=f"vaug{kt}", tag="va")
                nc.vector.tensor_copy(
                    out=va.rearrange("p (h c) -> p h c", c=65)[:, :, 64:65],
                    in_=ones_sb.rearrange("p (h one) -> p h one", one=1),
                )
                vaug_sb.append(va)

            # ---------- projection work units ----------
            def v_unit(r, tl, n):
                # V rows [512r+128tl, +128), output features [512n, +512)
                xbt_sb = x_chunk[r]
                kt = 4 * r + tl
                va3 = vaug_sb[kt].rearrange("p (h c) -> p h c", c=65)
                vps = ps_mm.tile([128, 512], f32, tag="mm")
                for k in range(NK):
                    nc.tensor.matmul(
                        out=vps[:],
                        lhsT=xbt_sb[k][:, 128 * tl : 128 * (tl + 1)],
                        rhs=wv_sb[n][k][:],
                        start=(k == 0),
                        stop=(k == NK - 1),
                    )
                nc.vector.tensor_add(
                    out=va3[:, 8 * n : 8 * (n + 1), 0:64],
                    in0=vps[:].rearrange("p (h c) -> p h c", c=64),
                    in1=bv_bc[:, 512 * n : 512 * (n + 1)].rearrange(
                        "p (h c) -> p h c", c=64
                    ),
                )

            def k_unit(r, m):
                # K^T slab m, keys [512r, +512)
                xbt_sb = x_chunk[r]
                kps = ps_mm.tile([128, 512], f32, tag="mm")
                for k in range(NK):
                    nc.tensor.matmul(
                        out=kps[:],
                        lhsT=wk_sb[m // 4][k][
                            :, 128 * (m % 4) : 128 * (m % 4 + 1)
                        ],
                        rhs=xbt_sb[k][:],
                        start=(k == 0),
                        stop=(k == NK - 1),
                    )
                nc.scalar.activation(
                    out=k2_sb[m][:, 512 * r : 512 * (r + 1)],
                    in_=kps[:],
                    func=ACT.Identity,
                    bias=bk_sb[:, m : m + 1],
                    scale=1.0,
                )

            def proj_units(r):
                return [(v_unit, (r, tl, n)) for n in range(2) for tl in range(4)] + [
                    (k_unit, (r, m)) for m in range(NK)
                ]

            # ---------- prologue: V/K of chunk 0, all of Q ----------
            for fn, args in proj_units(0):
                fn(*args)

            qp_sb = [None] * NK

            def q_unit(m):
                qps = ps_mm.tile([128, R], f32, tag="mm")
                for k in range(NK):
                    nc.tensor.matmul(
                        out=qps[:],
                        lhsT=wq_sb[k][:, 128 * m : 128 * (m + 1)],
                        rhs=x_chunk[0][k][:],
                        start=(k == 0),
                        stop=(k == NK - 1),
                    )
                qp = qt_pool.tile([128, R], bf16, name=f"qp{m}", tag="qt")
                nc.scalar.activation(
                    out=qp, in_=qps[:], func=ACT.Identity,
                    bias=bq_sb[:, m : m + 1], scale=1.0,
                )
                qp_sb[m] = qp

            # Q slabs 0-1 now; 2-7 ride sweep-0 filler slots (each >= 2
            # pairs ahead of its consumer)
            for m in range(2):
                q_unit(m)

            bo_bc = bias_pool.tile([128, D], f32, name="bo_bc", tag="bc")
            nc.gpsimd.dma_start(out=bo_bc, in_=bcast_row(bo))

            # cross-sweep attended accumulators (bf16) per (pair, head)
            aacc = [
                [
                    aacc_pool.tile([65, R], bf16, name=f"aacc{hp}_{o}", tag="aa")
                    for o in range(2)
                ]
                for hp in range(NHP)
            ]

            attT_sb = [
                att_pool.tile([128, R], bf16, name=f"attT{hp}", tag="att")
                for hp in range(NHP)
            ]

            def normalize(hp, o):
                den_sb = small.tile([1, R], f32, tag="densb")
                nc.vector.tensor_copy(out=den_sb, in_=aacc[hp][o][64:65, :])
                recip = small.tile([1, R], f32, tag="recip")
                nc.vector.reciprocal_approx_fast(out=recip, in_=den_sb)
                den = small.tile([64, R], f32, tag="den")
                nc.gpsimd.partition_broadcast(den, recip, channels=64)
                nc.vector.tensor_mul(
                    out=attT_sb[hp][64 * o : 64 * o + 64, :],
                    in0=aacc[hp][o][0:64, :],
                    in1=den,
                )

            # ---------- sweeps over key chunks ----------
            def emit_sps_exp(hp, kt):
                sps = ps_sc.tile([128, 2 * R], f32, tag="sc")
                for o in range(2):
                    nc.tensor.matmul(
                        out=sps[:, R * o : R * (o + 1)],
                        lhsT=k2_sb[hp][
                            64 * o : 64 * o + 64, 128 * kt : 128 * (kt + 1)
                        ],
                        rhs=qp_sb[hp][64 * o : 64 * o + 64, :],
                        start=True,
                        stop=True,
                    )
                ex = exp_pool.tile([128, 2 * R], bf16, tag="exp")
                nc.scalar.activation(
                    out=ex, in_=sps[:], func=ACT.Exp, bias=0.0, scale=SCALE,
                )
                return ex

            v_units = lambda r: [
                (v_unit, (r, tl, n)) for n in range(2) for tl in range(4)
            ]
            k_units = lambda r, ms: [(k_unit, (r, m)) for m in ms]
            units_by_sweep = [
                # sweep 0: Q slabs 2-7 lead (Q[m] lands >= 2 pairs before
                # pair m), then chunk-1 V (all needed by sweep-1 pair 0),
                # then chunk-1 K
                [(q_unit, (m,)) for m in range(2, NK)]
                + v_units(1) + k_units(1, range(NK)),
                v_units(2) + k_units(2, range(NK)),
                # chunk-3 K slabs 3-7 defer into sweep 3
                v_units(3) + k_units(3, range(3)),
                k_units(3, range(3, NK)),
            ]
            for r in range(RCH):
                units = units_by_sweep[r]
                ui = 0
                for hp in range(NHP):
                    exs = [emit_sps_exp(hp, 4 * r + j) for j in range(4)]
                    # projection filler while the Scalar engine runs the Exps.
                    # Sweep 3's K(3,m) units go one per pair (pair m-3), so
                    # each lands 3 pairs before its consumer.
                    quota = (
                        min(hp + 1, len(units))
                        if r == RCH - 1
                        else ((hp + 1) * len(units) + NHP - 1) // NHP
                    )
                    while ui < quota:
                        fn, args = units[ui]
                        fn(*args)
                        ui += 1
                    for o in range(2):
                        aph = ps_att.tile([65, R], f32, tag="att")
                        for j in range(4):
                            kt = 4 * r + j
                            h = 2 * hp + o
                            nc.tensor.matmul(
                                out=aph[:],
                                lhsT=vaug_sb[kt][:, 65 * h : 65 * h + 65],
                                rhs=exs[j][:, R * o : R * (o + 1)],
                                start=(j == 0),
                                stop=(j == 3),
                            )
                        acc = aacc[hp][o]
                        if r == 0:
                            nc.vector.tensor_copy(out=acc, in_=aph[:])
                        else:
                            nc.vector.tensor_add(out=acc, in0=acc, in1=aph[:])
                        if r == RCH - 1:
                            normalize(hp, o)
                assert ui == len(units)

            # ---------- output projection ----------
            for m in range(R // 128):
                for n in range(2):
                    ops = ps_mm.tile([128, 512], f32, tag="mm")
                    for k in range(NK):
                        nc.tensor.matmul(
                            out=ops[:],
                            lhsT=attT_sb[k][:, 128 * m : 128 * (m + 1)],
                            rhs=wo_sb[k][:, 512 * n : 512 * (n + 1)],
                            start=(k == 0),
                            stop=(k == NK - 1),
                        )
                    oev = outp.tile([128, 512], f32, tag="oev")
                    nc.vector.tensor_add(
                        out=oev, in0=ops[:], in1=bo_bc[:, 512 * n : 512 * (n + 1)]
                    )
                    nc.sync.dma_start(
                        out=out.ap()[128 * m : 128 * (m + 1), 512 * n : 512 * (n + 1)],
                        in_=oev,
                    )
    nc.finalize()
    return nc


def get_nc(use_f32r=True):
    key = use_f32r
    if key not in _COMPILED:
        _COMPILED[key] = build_nc(use_f32r)
    return _COMPILED[key]


def make_in_maps(x, Wq, bq, Wk, bk, Wv, bv, Wo, bo):
    bf = ml_dtypes.bfloat16
    x = np.asarray(x, np.float32)
    x_flat = x.reshape(B * S, D)
    weights = {
        "Wq": np.ascontiguousarray(np.asarray(Wq, np.float32).astype(bf)),
        "Wk": np.ascontiguousarray(np.asarray(Wk, np.float32).astype(bf)),
        "Wv": np.ascontiguousarray(np.asarray(Wv, np.float32).astype(bf)),
        "Wo": np.ascontiguousarray(np.asarray(Wo, np.float32).astype(bf)),
        "bq": np.asarray(bq, np.float32).reshape(D, 1),
        "bk": np.asarray(bk, np.float32).reshape(D, 1),
        "bv": np.asarray(bv, np.float32).reshape(D, 1),
        "bo": np.asarray(bo, np.float32).reshape(D, 1),
    }
    x_bf = x_flat.astype(bf)
    in_maps = []
    cpb = NCORES // B  # cores per batch
    for c in range(NCORES):
        b = c // cpb
        xb = x_bf[S * b : S * (b + 1), :]  # [2048, 1024] this core's batch
        # permute rows so this core's own 512 rows come first
        j = c % cpb
        perm = np.concatenate(
            [xb[512 * j : 512 * (j + 1)]]
            + [xb[512 * i : 512 * (i + 1)] for i in range(cpb) if i != j]
        )
        in_maps.append({"xTb": np.ascontiguousarray(perm.T), **weights})
    return in_maps


def gather_out(results):
    outs = [results[c]["out"] for c in range(NCORES)]
    return np.concatenate(outs, axis=0).reshape(B, S, D)


def kernel(x, Wq, bq, Wk, bk, Wv, bv, Wo, bo, _use_f32r=True):
    in_maps = make_in_maps(x, Wq, bq, Wk, bk, Wv, bv, Wo, bo)
    nc = get_nc(_use_f32r)
    res = run_bass_kernel_spmd(nc, in_maps, list(range(NCORES)))
    return gather_out(res.results)


# revision 37
# speedup vs baseline: 1.0141x; 1.0141x over previous
"""Distributed multi-head attention layer on 8 TRN2 NeuronCores.

Problem: B=2, S=2048, D=1024, H=16 heads, head_dim=64, fp32.

Sharding: sequence-parallel over query rows, with REDUNDANT per-core
K/V computation instead of collectives (a 4-rank AllGather costs
~15us + size/40GBps here — far more than the extra matmuls). The
4096 global rows split into 8 chunks of 512; core c owns query rows
[512c, 512c+512), all in batch b = c // 4. The host permutes each
core's batch so its own 512 rows come first (attention is
permutation-invariant over keys). Q^T and the output projection
cover only chunk 0; K^T/V cover all 4 chunks of the batch.

Schedule: prologue projects V/K of chunk 0 and Q^T slabs 0-1; then 4
"sweeps", one per key chunk. Q^T slabs 2-7 ride sweep-0 filler slots
and chunk-3 K slabs 3-7 defer into sweep 3, so the DMA-gated prologue
stays short and the filler-less last sweep is not Scalar-starved.
Weight DMAs stream in consumption order, each spread across all three
DMA queues, and the x ring holds two chunks so chunk r+1's load
overlaps sweep r-1. Sweep r runs attention for all 8 head
pairs over chunk r's 4 key tiles, with the V/K projections of chunk
r+1 interleaved between head pairs so the PE stays busy while the
Scalar engine computes the softmax Exps. Attended values accumulate
in PSUM within a sweep (4 contiguous matmuls — interrupting an
accumulation group costs ~2x on the resume) and in an SBUF bf16
accumulator across sweeps (DVE add).

Per (pair, key-tile): both heads' scores land in one [128,1024] PSUM
tile via two matmuls on PE row groups 0/64 (which execute
concurrently), one Exp activation covers both, and the ones-column of
the V tiles produces the softmax denominator inside the attended
matmul. Normalization (broadcast fast-reciprocal multiply) is emitted
inside sweep 3 right after each pair's last accumulate, so the 16
chains overlap sweep-3 compute and the hoisted output projection.

Empirically rejected variants (each re-measured at +55us): K/Q bias
adds on DVE instead of Scalar (DVE back-pressure stalls PSUM slot
release), mid-sweep x-chunk DMA dispatch moved off the Scalar queue,
and a DRAM AllGather for K/V (15us + size/40GBps makes remote K/V
arrive far too late; redundant compute wins).
"""

import sys

sys.path.insert(0, "/opt/trn_rl_repo")

import ml_dtypes
import numpy as np

import concourse.bass as bass
import concourse.tile as tile
from concourse import bacc, mybir
from concourse.bass_utils import run_bass_kernel_spmd

f32 = mybir.dt.float32
f32r = mybir.dt.float32r
bf16 = mybir.dt.bfloat16
ACT = mybir.ActivationFunctionType

B, S, D = 2, 2048, 1024
H, HD = 16, 64
NCORES = 8
R = (B * S) // NCORES  # 512 query rows per core
NK = D // 128  # 8 contraction tiles
NKT = S // 128  # 16 key tiles per batch
RCH = S // 512  # 4 row-chunks of the batch
NHP = H // 2  # 8 head pairs
SCALE = 1.0 / float(np.sqrt(HD))

_COMPILED = {}


def build_nc(use_f32r=True):
    nc = bacc.Bacc("TRN2", target_bir_lowering=False, debug=False, num_devices=8)

    xTb = nc.dram_tensor("xTb", [D, S], bf16, kind="ExternalInput")
    Wq = nc.dram_tensor("Wq", [D, D], bf16, kind="ExternalInput")
    Wk = nc.dram_tensor("Wk", [D, D], bf16, kind="ExternalInput")
    Wv = nc.dram_tensor("Wv", [D, D], bf16, kind="ExternalInput")
    Wo = nc.dram_tensor("Wo", [D, D], bf16, kind="ExternalInput")
    bq = nc.dram_tensor("bq", [D, 1], f32, kind="ExternalInput")
    bk = nc.dram_tensor("bk", [D, 1], f32, kind="ExternalInput")
    bv = nc.dram_tensor("bv", [D, 1], f32, kind="ExternalInput")
    bo = nc.dram_tensor("bo", [D, 1], f32, kind="ExternalInput")
    out = nc.dram_tensor("out", [R, D], f32, kind="ExternalOutput")

    def bcast_row(handle):
        # [D,1] bias -> [128, D] partition-broadcast DMA source
        return bass.AP(tensor=handle.ap().tensor, offset=0, ap=[[0, 128], [1, D]])

    with tile.TileContext(nc) as tc:
        with (
            # wk/wv resident through sweep 2; wq only for the prologue
            tc.tile_pool(name="wpool", bufs=24) as wpool,
            tc.tile_pool(name="wopool", bufs=8) as wopool,
            # streamed x^T tiles, 2-chunk ring: chunk r+2's DMA starts as
            # soon as chunk r's last reader retires
            tc.tile_pool(name="xbt", bufs=16) as xbt_pool,
            # K^T of the whole batch, bf16, feature-major: 8 x [128, 2048]
            tc.tile_pool(name="k2", bufs=8) as k2_pool,
            # ones-augmented V of the whole batch, bf16: 16 x [128, 16*65]
            tc.tile_pool(name="vaug", bufs=16) as vaug_pool,
            tc.tile_pool(name="qt_pool", bufs=8) as qt_pool,
            # cross-sweep attended accumulators, bf16: 16 x [65, 512]
            tc.tile_pool(name="aacc", bufs=16) as aacc_pool,
            tc.tile_pool(name="att_pool", bufs=8) as att_pool,
            tc.tile_pool(name="exp_pool", bufs=5) as exp_pool,
            tc.tile_pool(name="outp", bufs=2) as outp,
            tc.tile_pool(name="bias", bufs=1) as bias_pool,
            tc.tile_pool(name="small", bufs=2) as small,
            tc.tile_pool(name="ps_mm", bufs=2, space="PSUM") as ps_mm,
            tc.tile_pool(name="ps_sc", bufs=2, space="PSUM") as ps_sc,
            tc.tile_pool(name="ps_att", bufs=2, space="PSUM") as ps_att,
        ):
            dma_round = [nc.sync, nc.scalar, nc.gpsimd]

            def load_chunk(r):
                tiles = []
                for k in range(NK):
                    xb = xbt_pool.tile([128, 512], bf16, name=f"xb{r}_{k}", tag="xb")
                    dma_round[k % 3].dma_start(
                        out=xb,
                        in_=xTb.ap()[
                            128 * k : 128 * (k + 1), 512 * r : 512 * (r + 1)
                        ],
                    )
                    tiles.append(xb)
                return tiles

            # weights stream in consumption order, each spread across all
            # three DMA queues so the first projection units unblock early:
            # Wv (V units lead the prologue), chunk 0, Wk, Wq (sweep-0
            # fillers), Wo (output projection)
            def load_w(pool, tag, dram, k, dt=bf16):
                t = pool.tile([128, D], dt, name=f"{tag}{k}", tag=tag[0] if tag != "wo" else "wo")
                dma_round[k % 3].dma_start(out=t, in_=dram.ap()[128 * k : 128 * (k + 1), :])
                return t

            wv_sb = [load_w(wpool, "wv", Wv, k) for k in range(NK)]
            x_chunk = [load_chunk(0)]
            wk_sb = [load_w(wpool, "wk", Wk, k) for k in range(NK)]
            x_chunk += [load_chunk(r) for r in range(1, RCH)]
            wq_sb = [load_w(wpool, "wq", Wq, k) for k in range(NK)]
            wo_sb = [load_w(wopool, "wo", Wo, k) for k in range(NK)]

            bk_sb = bias_pool.tile([128, NK], f32)
            nc.gpsimd.dma_start(
                out=bk_sb, in_=bk.ap().rearrange("(k p) one -> p k one", p=128)
            )
            bq_sb = bias_pool.tile([128, NK], f32)
            nc.gpsimd.dma_start(
                out=bq_sb, in_=bq.ap().rearrange("(k p) one -> p k one", p=128)
            )
            bv_bc = bias_pool.tile([128, D], f32, name="bv_bc", tag="bc")
            nc.gpsimd.dma_start(out=bv_bc, in_=bcast_row(bv))

            k2_sb = [
                k2_pool.tile([128, S], bf16, name=f"k2_{m}", tag="k2")
                for m in range(NK)
            ]
            ones_dram = nc.inline_tensor(
                np.ones((1, H), ml_dtypes.bfloat16), name="ones16"
            )
            ones_sb = bias_pool.tile([128, H], bf16, name="ones_sb", tag="ones")
            nc.gpsimd.dma_start(
                out=ones_sb,
                in_=bass.AP(tensor=ones_dram, offset=0, ap=[[0, 128], [1, H]]),
            )
            vaug_sb = []
            for kt in range(NKT):
                va = vaug_pool.tile([128, H * 65], bf16, name=f"vaug{kt}", tag="va")
                nc.vector.tensor_copy(
                    out=va.rearrange("p (h c) -> p h c", c=65)[:, :, 64:65],
                    in_=ones_sb.rearrange("p (h one) -> p h one", one=1),
                )
                vaug_sb.append(va)

            # ---------- projection work units ----------
            def v_unit(r, tl, n):
                # V rows [512r+128tl, +128), output features [512n, +512)
                xbt_sb = x_chunk[r]
                kt = 4 * r + tl
                va3 = vaug_sb[kt].rearrange("p (h c) -> p h c", c=65)
                vps = ps_mm.tile([128, 512], f32, tag="mm")
                for k in range(NK):
                    nc.tensor.matmul(
                        out=vps[:],
                        lhsT=xbt_sb[k][:, 128 * tl : 128 * (tl + 1)],
                        rhs=wv_sb[k][:, 512 * n : 512 * (n + 1)],
                        start=(k == 0),
                        stop=(k == NK - 1),
                    )
                nc.vector.tensor_add(
                    out=va3[:, 8 * n : 8 * (n + 1), 0:64],
                    in0=vps[:].rearrange("p (h c) -> p h c", c=64),
                    in1=bv_bc[:, 512 * n : 512 * (n + 1)].rearrange(
                        "p (h c) -> p h c", c=64
                    ),
                )

            def k_unit(r, m):
                # K^T slab m, keys [512r, +512)
                xbt_sb = x_chunk[r]
                kps = ps_mm.tile([128, 512], f32, tag="mm")
                for k in range(NK):
                    nc.tensor.matmul(
                        out=kps[:],
                        lhsT=wk_sb[k][:, 128 * m : 128 * (m + 1)],
                        rhs=xbt_sb[k][:],
                        start=(k == 0),
                        stop=(k == NK - 1),
                    )
                nc.scalar.activation(
                    out=k2_sb[m][:, 512 * r : 512 * (r + 1)],
                    in_=kps[:],
                    func=ACT.Identity,
                    bias=bk_sb[:, m : m + 1],
                    scale=1.0,
                )

            def proj_units(r):
                return [(v_unit, (r, tl, n)) for tl in range(4) for n in range(2)] + [
                    (k_unit, (r, m)) for m in range(NK)
                ]

            # ---------- prologue: V/K of chunk 0, all of Q ----------
            for fn, args in proj_units(0):
                fn(*args)

            qp_sb = [None] * NK

            def q_unit(m):
                qps = ps_mm.tile([128, R], f32, tag="mm")
                for k in range(NK):
                    nc.tensor.matmul(
                        out=qps[:],
                        lhsT=wq_sb[k][:, 128 * m : 128 * (m + 1)],
                        rhs=x_chunk[0][k][:],
                        start=(k == 0),
                        stop=(k == NK - 1),
                    )
                qp = qt_pool.tile([128, R], bf16, name=f"qp{m}", tag="qt")
                nc.scalar.activation(
                    out=qp, in_=qps[:], func=ACT.Identity,
                    bias=bq_sb[:, m : m + 1], scale=1.0,
                )
                qp_sb[m] = qp

            # Q slabs 0-1 now; 2-7 ride sweep-0 filler slots (each >= 2
            # pairs ahead of its consumer)
            for m in range(2):
                q_unit(m)

            bo_bc = bias_pool.tile([128, D], f32, name="bo_bc", tag="bc")
            nc.gpsimd.dma_start(out=bo_bc, in_=bcast_row(bo))

            # cross-sweep attended accumulators (bf16) per (pair, head)
            aacc = [
                [
                    aacc_pool.tile([65, R], bf16, name=f"aacc{hp}_{o}", tag="aa")
                    for o in range(2)
                ]
                for hp in range(NHP)
            ]

            attT_sb = [
                att_pool.tile([128, R], bf16, name=f"attT{hp}", tag="att")
                for hp in range(NHP)
            ]

            def normalize(hp, o):
                den_sb = small.tile([1, R], f32, tag="densb")
                nc.vector.tensor_copy(out=den_sb, in_=aacc[hp][o][64:65, :])
                recip = small.tile([1, R], f32, tag="recip")
                nc.vector.reciprocal_approx_fast(out=recip, in_=den_sb)
                den = small.tile([64, R], f32, tag="den")
                nc.gpsimd.partition_broadcast(den, recip, channels=64)
                nc.vector.tensor_mul(
                    out=attT_sb[hp][64 * o : 64 * o + 64, :],
                    in0=aacc[hp][o][0:64, :],
                    in1=den,
                )

            # ---------- sweeps over key chunks ----------
            def emit_sps_exp(hp, kt):
                sps = ps_sc.tile([128, 2 * R], f32, tag="sc")
                for o in range(2):
                    nc.tensor.matmul(
                        out=sps[:, R * o : R * (o + 1)],
                        lhsT=k2_sb[hp][
                            64 * o : 64 * o + 64, 128 * kt : 128 * (kt + 1)
                        ],
                        rhs=qp_sb[hp][64 * o : 64 * o + 64, :],
                        start=True,
                        stop=True,
                    )
                ex = exp_pool.tile([128, 2 * R], bf16, tag="exp")
                nc.scalar.activation(
                    out=ex, in_=sps[:], func=ACT.Exp, bias=0.0, scale=SCALE,
                )
                return ex

            v_units = lambda r: [
                (v_unit, (r, tl, n)) for tl in range(4) for n in range(2)
            ]
            k_units = lambda r, ms: [(k_unit, (r, m)) for m in ms]
            units_by_sweep = [
                # sweep 0: Q slabs 2-7 lead (Q[m] lands >= 2 pairs before
                # pair m), then chunk-1 V (all needed by sweep-1 pair 0),
                # then chunk-1 K
                [(q_unit, (m,)) for m in range(2, NK)]
                + v_units(1) + k_units(1, range(NK)),
                v_units(2) + k_units(2, range(NK)),
                # chunk-3 K slabs 3-7 defer into sweep 3
                v_units(3) + k_units(3, range(3)),
                k_units(3, range(3, NK)),
            ]
            for r in range(RCH):
                units = units_by_sweep[r]
                ui = 0
                for hp in range(NHP):
                    exs = [emit_sps_exp(hp, 4 * r + j) for j in range(4)]
                    # projection filler while the Scalar engine runs the Exps.
                    # Sweep 3's K(3,m) units go one per pair (pair m-3), so
                    # each lands 3 pairs before its consumer.
                    quota = (
                        min(hp + 1, len(units))
                        if r == RCH - 1
                        else ((hp + 1) * len(units) + NHP - 1) // NHP
                    )
                    while ui < quota:
                        fn, args = units[ui]
                        fn(*args)
                        ui += 1
                    for o in range(2):
                        aph = ps_att.tile([65, R], f32, tag="att")
                        for j in range(4):
                            kt = 4 * r + j
                            h = 2 * hp + o
                            nc.tensor.matmul(
                                out=aph[:],
                                lhsT=vaug_sb[kt][:, 65 * h : 65 * h + 65],
                                rhs=exs[j][:, R * o : R * (o + 1)],
                                start=(j == 0),
                                stop=(j == 3),
                            )
                        acc = aacc[hp][o]
                        if r == 0:
                            nc.vector.tensor_copy(out=acc, in_=aph[:])
                        else:
                            nc.vector.tensor_add(out=acc, in0=acc, in1=aph[:])
                        if r == RCH - 1:
                            normalize(hp, o)
                assert ui == len(units)

            # ---------- output projection ----------
            for m in range(R // 128):
                for n in range(2):
                    ops = ps_mm.tile([128, 512], f32, tag="mm")
                    for k in range(NK):
                        nc.tensor.matmul(
                            out=ops[:],
                            lhsT=attT_sb[k][:, 128 * m : 128 * (m + 1)],
                            rhs=wo_sb[k][:, 512 * n : 512 * (n + 1)],
                            start=(k == 0),
                            stop=(k == NK - 1),
                        )
                    oev = outp.tile([128, 512], f32, tag="oev")
                    nc.vector.tensor_add(
                        out=oev, in0=ops[:], in1=bo_bc[:, 512 * n : 512 * (n + 1)]
                    )
                    nc.sync.dma_start(
                        out=out.ap()[128 * m : 128 * (m + 1), 512 * n : 512 * (n + 1)],
                        in_=oev,
                    )
    nc.finalize()
    return nc


def get_nc(use_f32r=True):
    key = use_f32r
    if key not in _COMPILED:
        _COMPILED[key] = build_nc(use_f32r)
    return _COMPILED[key]


def make_in_maps(x, Wq, bq, Wk, bk, Wv, bv, Wo, bo):
    bf = ml_dtypes.bfloat16
    x = np.asarray(x, np.float32)
    x_flat = x.reshape(B * S, D)
    weights = {
        "Wq": np.ascontiguousarray(np.asarray(Wq, np.float32).astype(bf)),
        "Wk": np.ascontiguousarray(np.asarray(Wk, np.float32).astype(bf)),
        "Wv": np.ascontiguousarray(np.asarray(Wv, np.float32).astype(bf)),
        "Wo": np.ascontiguousarray(np.asarray(Wo, np.float32).astype(bf)),
        "bq": np.asarray(bq, np.float32).reshape(D, 1),
        "bk": np.asarray(bk, np.float32).reshape(D, 1),
        "bv": np.asarray(bv, np.float32).reshape(D, 1),
        "bo": np.asarray(bo, np.float32).reshape(D, 1),
    }
    x_bf = x_flat.astype(bf)
    in_maps = []
    cpb = NCORES // B  # cores per batch
    for c in range(NCORES):
        b = c // cpb
        xb = x_bf[S * b : S * (b + 1), :]  # [2048, 1024] this core's batch
        # permute rows so this core's own 512 rows come first
        j = c % cpb
        perm = np.concatenate(
            [xb[512 * j : 512 * (j + 1)]]
            + [xb[512 * i : 512 * (i + 1)] for i in range(cpb) if i != j]
        )
        in_maps.append({"xTb": np.ascontiguousarray(perm.T), **weights})
    return in_maps


def gather_out(results):
    outs = [results[c]["out"] for c in range(NCORES)]
    return np.concatenate(outs, axis=0).reshape(B, S, D)


def kernel(x, Wq, bq, Wk, bk, Wv, bv, Wo, bo, _use_f32r=True):
    in_maps = make_in_maps(x, Wq, bq, Wk, bk, Wv, bv, Wo, bo)
    nc = get_nc(_use_f32r)
    res = run_bass_kernel_spmd(nc, in_maps, list(range(NCORES)))
    return gather_out(res.results)


# revision 39
# speedup vs baseline: 1.0233x; 1.0090x over previous
"""Distributed multi-head attention layer on 8 TRN2 NeuronCores.

Problem: B=2, S=2048, D=1024, H=16 heads, head_dim=64, fp32.

Sharding: sequence-parallel over query rows, with REDUNDANT per-core
K/V computation instead of collectives (a 4-rank AllGather costs
~15us + size/40GBps here — far more than the extra matmuls). The
4096 global rows split into 8 chunks of 512; core c owns query rows
[512c, 512c+512), all in batch b = c // 4. The host permutes each
core's batch so its own 512 rows come first (attention is
permutation-invariant over keys). Q^T and the output projection
cover only chunk 0; K^T/V cover all 4 chunks of the batch.

Schedule: prologue projects V/K of chunk 0 and Q^T slabs 0-1; then 4
"sweeps", one per key chunk. Q^T slabs 2-7 ride sweep-0 filler slots
and chunk-3 K slabs 3-7 defer into sweep 3, so the DMA-gated prologue
stays short and the filler-less last sweep is not Scalar-starved.
Weight DMAs stream in consumption order, each spread across all three
DMA queues, and the x ring holds two chunks so chunk r+1's load
overlaps sweep r-1. Sweep r runs attention for all 8 head
pairs over chunk r's 4 key tiles, with the V/K projections of chunk
r+1 interleaved between head pairs so the PE stays busy while the
Scalar engine computes the softmax Exps. Attended values accumulate
in PSUM within a sweep (4 contiguous matmuls — interrupting an
accumulation group costs ~2x on the resume) and in an SBUF bf16
accumulator across sweeps (DVE add).

Per (pair, key-tile): both heads' scores land in one [128,1024] PSUM
tile via two matmuls on PE row groups 0/64 (which execute
concurrently), one Exp activation covers both, and the ones-column of
the V tiles produces the softmax denominator inside the attended
matmul. Normalization (broadcast fast-reciprocal multiply) is emitted
inside sweep 3 right after each pair's last accumulate, so the 16
chains overlap sweep-3 compute and the hoisted output projection.

Empirically rejected variants (each re-measured at +55us): K/Q bias
adds on DVE instead of Scalar (DVE back-pressure stalls PSUM slot
release), mid-sweep x-chunk DMA dispatch moved off the Scalar queue,
and a DRAM AllGather for K/V (15us + size/40GBps makes remote K/V
arrive far too late; redundant compute wins).
"""

import sys

sys.path.insert(0, "/opt/trn_rl_repo")

import ml_dtypes
import numpy as np

import concourse.bass as bass
import concourse.tile as tile
from concourse import bacc, mybir
from concourse.bass_utils import run_bass_kernel_spmd

f32 = mybir.dt.float32
f32r = mybir.dt.float32r
bf16 = mybir.dt.bfloat16
ACT = mybir.ActivationFunctionType

B, S, D = 2, 2048, 1024
H, HD = 16, 64
NCORES = 8
R = (B * S) // NCORES  # 512 query rows per core
NK = D // 128  # 8 contraction tiles
NKT = S // 128  # 16 key tiles per batch
RCH = S // 512  # 4 row-chunks of the batch
NHP = H // 2  # 8 head pairs
SCALE = 1.0 / float(np.sqrt(HD))

_COMPILED = {}


def build_nc(use_f32r=True):
    nc = bacc.Bacc("TRN2", target_bir_lowering=False, debug=False, num_devices=8)

    xTb = nc.dram_tensor("xTb", [D, S], bf16, kind="ExternalInput")
    Wq = nc.dram_tensor("Wq", [D, D], bf16, kind="ExternalInput")
    Wk = nc.dram_tensor("Wk", [D, D], bf16, kind="ExternalInput")
    Wv = nc.dram_tensor("Wv", [D, D], bf16, kind="ExternalInput")
    Wo = nc.dram_tensor("Wo", [D, D], bf16, kind="ExternalInput")
    bq = nc.dram_tensor("bq", [D, 1], f32, kind="ExternalInput")
    bk = nc.dram_tensor("bk", [D, 1], f32, kind="ExternalInput")
    bv = nc.dram_tensor("bv", [D, 1], f32, kind="ExternalInput")
    bo = nc.dram_tensor("bo", [D, 1], f32, kind="ExternalInput")
    out = nc.dram_tensor("out", [R, D], f32, kind="ExternalOutput")

    def bcast_row(handle):
        # [D,1] bias -> [128, D] partition-broadcast DMA source
        return bass.AP(tensor=handle.ap().tensor, offset=0, ap=[[0, 128], [1, D]])

    with tile.TileContext(nc) as tc:
        with (
            # wk/wv resident through sweep 2; wq only for the prologue
            tc.tile_pool(name="wpool", bufs=24) as wpool,
            tc.tile_pool(name="wopool", bufs=8) as wopool,
            # streamed x^T tiles, 2-chunk ring: chunk r+2's DMA starts as
            # soon as chunk r's last reader retires
            tc.tile_pool(name="xbt", bufs=16) as xbt_pool,
            # K^T of the whole batch, bf16, feature-major: 8 x [128, 2048]
            tc.tile_pool(name="k2", bufs=8) as k2_pool,
            # ones-augmented V of the whole batch, bf16: 16 x [128, 16*65]
            tc.tile_pool(name="vaug", bufs=16) as vaug_pool,
            tc.tile_pool(name="qt_pool", bufs=8) as qt_pool,
            # cross-sweep attended accumulators, bf16: 16 x [65, 512]
            tc.tile_pool(name="aacc", bufs=16) as aacc_pool,
            tc.tile_pool(name="att_pool", bufs=8) as att_pool,
            tc.tile_pool(name="exp_pool", bufs=5) as exp_pool,
            tc.tile_pool(name="outp", bufs=2) as outp,
            tc.tile_pool(name="bias", bufs=1) as bias_pool,
            tc.tile_pool(name="small", bufs=2) as small,
            tc.tile_pool(name="ps_mm", bufs=2, space="PSUM") as ps_mm,
            tc.tile_pool(name="ps_sc", bufs=2, space="PSUM") as ps_sc,
            tc.tile_pool(name="ps_att", bufs=2, space="PSUM") as ps_att,
        ):
            dma_round = [nc.sync, nc.scalar, nc.gpsimd]

            def load_chunk(r):
                tiles = []
                for k in range(NK):
                    xb = xbt_pool.tile([128, 512], bf16, name=f"xb{r}_{k}", tag="xb")
                    dma_round[k % 3].dma_start(
                        out=xb,
                        in_=xTb.ap()[
                            128 * k : 128 * (k + 1), 512 * r : 512 * (r + 1)
                        ],
                    )
                    tiles.append(xb)
                return tiles

            # weights stream in consumption order, each spread across all
            # three DMA queues so the first projection units unblock early:
            # Wv (V units lead the prologue), chunk 0, Wk, Wq (sweep-0
            # fillers), Wo (output projection)
            def load_w(pool, tag, dram, k, dt=bf16):
                t = pool.tile([128, D], dt, name=f"{tag}{k}", tag=tag[0] if tag != "wo" else "wo")
                dma_round[k % 3].dma_start(out=t, in_=dram.ap()[128 * k : 128 * (k + 1), :])
                return t

            wv_sb = [load_w(wpool, "wv", Wv, k) for k in range(NK)]
            x_chunk = [load_chunk(0)]
            wk_sb = [load_w(wpool, "wk", Wk, k) for k in range(NK)]
            x_chunk += [load_chunk(r) for r in range(1, RCH)]
            wq_sb = [load_w(wpool, "wq", Wq, k) for k in range(NK)]
            wo_sb = [load_w(wopool, "wo", Wo, k) for k in range(NK)]

            bk_sb = bias_pool.tile([128, NK], f32)
            nc.gpsimd.dma_start(
                out=bk_sb, in_=bk.ap().rearrange("(k p) one -> p k one", p=128)
            )
            bq_sb = bias_pool.tile([128, NK], f32)
            nc.gpsimd.dma_start(
                out=bq_sb, in_=bq.ap().rearrange("(k p) one -> p k one", p=128)
            )
            bv_bc = bias_pool.tile([128, D], f32, name="bv_bc", tag="bc")
            nc.gpsimd.dma_start(out=bv_bc, in_=bcast_row(bv))

            k2_sb = [
                k2_pool.tile([128, S], bf16, name=f"k2_{m}", tag="k2")
                for m in range(NK)
            ]
            ones_dram = nc.inline_tensor(
                np.ones((1, H), ml_dtypes.bfloat16), name="ones16"
            )
            ones_sb = bias_pool.tile([128, H], bf16, name="ones_sb", tag="ones")
            nc.gpsimd.dma_start(
                out=ones_sb,
                in_=bass.AP(tensor=ones_dram, offset=0, ap=[[0, 128], [1, H]]),
            )
            vaug_sb = []
            for kt in range(NKT):
                va = vaug_pool.tile([128, H * 65], bf16, name=f"vaug{kt}", tag="va")
                nc.vector.tensor_copy(
                    out=va.rearrange("p (h c) -> p h c", c=65)[:, :, 64:65],
                    in_=ones_sb.rearrange("p (h one) -> p h one", one=1),
                )
                vaug_sb.append(va)

            # ---------- projection work units ----------
            def v_unit(r, tl, n):
                # V rows [512r+128tl, +128), output features [512n, +512)
                xbt_sb = x_chunk[r]
                kt = 4 * r + tl
                va3 = vaug_sb[kt].rearrange("p (h c) -> p h c", c=65)
                vps = ps_mm.tile([128, 512], f32, tag="mm")
                for k in range(NK):
                    nc.tensor.matmul(
                        out=vps[:],
                        lhsT=xbt_sb[k][:, 128 * tl : 128 * (tl + 1)],
                        rhs=wv_sb[k][:, 512 * n : 512 * (n + 1)],
                        start=(k == 0),
                        stop=(k == NK - 1),
                    )
                nc.vector.tensor_add(
                    out=va3[:, 8 * n : 8 * (n + 1), 0:64],
                    in0=vps[:].rearrange("p (h c) -> p h c", c=64),
                    in1=bv_bc[:, 512 * n : 512 * (n + 1)].rearrange(
                        "p (h c) -> p h c", c=64
                    ),
                )

            def k_unit(r, m):
                # K^T slab m, keys [512r, +512)
                xbt_sb = x_chunk[r]
                kps = ps_mm.tile([128, 512], f32, tag="mm")
                for k in range(NK):
                    nc.tensor.matmul(
                        out=kps[:],
                        lhsT=wk_sb[k][:, 128 * m : 128 * (m + 1)],
                        rhs=xbt_sb[k][:],
                        start=(k == 0),
                        stop=(k == NK - 1),
                    )
                nc.scalar.activation(
                    out=k2_sb[m][:, 512 * r : 512 * (r + 1)],
                    in_=kps[:],
                    func=ACT.Identity,
                    bias=bk_sb[:, m : m + 1],
                    scale=1.0,
                )

            def proj_units(r):
                return [(v_unit, (r, tl, n)) for tl in range(4) for n in range(2)] + [
                    (k_unit, (r, m)) for m in range(NK)
                ]

            # ---------- prologue: V/K of chunk 0, all of Q ----------
            for fn, args in proj_units(0):
                fn(*args)

            qp_sb = [None] * NK

            def q_unit(m):
                qps = ps_mm.tile([128, R], f32, tag="mm")
                for k in range(NK):
                    nc.tensor.matmul(
                        out=qps[:],
                        lhsT=wq_sb[k][:, 128 * m : 128 * (m + 1)],
                        rhs=x_chunk[0][k][:],
                        start=(k == 0),
                        stop=(k == NK - 1),
                    )
                qp = qt_pool.tile([128, R], bf16, name=f"qp{m}", tag="qt")
                nc.scalar.activation(
                    out=qp, in_=qps[:], func=ACT.Identity,
                    bias=bq_sb[:, m : m + 1], scale=1.0,
                )
                qp_sb[m] = qp

            # Q slabs 0-1 now; 2-7 ride sweep-0 filler slots (each >= 2
            # pairs ahead of its consumer)
            for m in range(2):
                q_unit(m)

            bo_bc = bias_pool.tile([128, D], f32, name="bo_bc", tag="bc")
            nc.gpsimd.dma_start(out=bo_bc, in_=bcast_row(bo))

            # cross-sweep attended accumulators (bf16) per (pair, head)
            aacc = [
                [
                    aacc_pool.tile([65, R], bf16, name=f"aacc{hp}_{o}", tag="aa")
                    for o in range(2)
                ]
                for hp in range(NHP)
            ]

            attT_sb = [
                att_pool.tile([128, R], bf16, name=f"attT{hp}", tag="att")
                for hp in range(NHP)
            ]

            def normalize(hp, o):
                den_sb = small.tile([1, R], f32, tag="densb")
                nc.vector.tensor_copy(out=den_sb, in_=aacc[hp][o][64:65, :])
                recip = small.tile([1, R], f32, tag="recip")
                nc.vector.reciprocal_approx_fast(out=recip, in_=den_sb)
                den = small.tile([64, R], f32, tag="den")
                nc.gpsimd.partition_broadcast(den, recip, channels=64)
                nc.vector.tensor_mul(
                    out=attT_sb[hp][64 * o : 64 * o + 64, :],
                    in0=aacc[hp][o][0:64, :],
                    in1=den,
                )

            # ---------- sweeps over key chunks ----------
            def emit_sps_exp(hp, kt):
                sps = ps_sc.tile([128, 2 * R], f32, tag="sc")
                for o in range(2):
                    nc.tensor.matmul(
                        out=sps[:, R * o : R * (o + 1)],
                        lhsT=k2_sb[hp][
                            64 * o : 64 * o + 64, 128 * kt : 128 * (kt + 1)
                        ],
                        rhs=qp_sb[hp][64 * o : 64 * o + 64, :],
                        start=True,
                        stop=True,
                    )
                ex = exp_pool.tile([128, 2 * R], bf16, tag="exp")
                nc.scalar.activation(
                    out=ex, in_=sps[:], func=ACT.Exp, bias=0.0, scale=SCALE,
                )
                return ex

            v_units = lambda r: [
                (v_unit, (r, tl, n)) for tl in range(4) for n in range(2)
            ]
            k_units = lambda r, ms: [(k_unit, (r, m)) for m in ms]
            units_by_sweep = [
                # sweep 0: Q slabs 2-7 lead (Q[m] lands >= 2 pairs before
                # pair m), then chunk-1 V (all needed by sweep-1 pair 0),
                # then chunk-1 K
                [(q_unit, (m,)) for m in range(2, NK)]
                + v_units(1) + k_units(1, range(NK)),
                v_units(2) + k_units(2, range(NK)),
                # chunk-3 K slabs 3-7 defer into sweep 3
                v_units(3) + k_units(3, range(3)),
                k_units(3, range(3, NK)),
            ]
            for r in range(RCH):
                units = units_by_sweep[r]
                ui = 0
                for hp in range(NHP):
                    exs = [emit_sps_exp(hp, 4 * r + j) for j in range(4)]
                    # projection filler while the Scalar engine runs the Exps.
                    # Sweep 3's K(3,m) units go one per pair (pair m-3), so
                    # each lands 3 pairs before its consumer.
                    quota = (
                        min(hp + 1, len(units))
                        if r == RCH - 1
                        else ((hp + 1) * len(units) + NHP - 1) // NHP
                    )
                    while ui < quota:
                        fn, args = units[ui]
                        fn(*args)
                        ui += 1
                    for o in range(2):
                        aph = ps_att.tile([65, R], f32, tag="att")
                        for j in range(4):
                            kt = 4 * r + j
                            h = 2 * hp + o
                            nc.tensor.matmul(
                                out=aph[:],
                                lhsT=vaug_sb[kt][:, 65 * h : 65 * h + 65],
                                rhs=exs[j][:, R * o : R * (o + 1)],
                                start=(j == 0),
                                stop=(j == 3),
                            )
                        acc = aacc[hp][o]
                        if r == 0:
                            nc.vector.tensor_copy(out=acc, in_=aph[:])
                        else:
                            nc.vector.tensor_add(out=acc, in0=acc, in1=aph[:])
                        if r == RCH - 1:
                            normalize(hp, o)
                assert ui == len(units)

            # ---------- output projection ----------
            for m in range(R // 128):
                for n in range(2):
                    # alternate PSUM pools: ps_att's banks are free after the
                    # last attended accumulate, so 4 output groups (not 2)
                    # can pre-accumulate k<7 before the final attT lands
                    ops = (
                        ps_mm.tile([128, 512], f32, tag="mm", name=f"ops{m}")
                        if n == 0
                        else ps_att.tile([128, 512], f32, tag="att", name=f"opsb{m}")
                    )
                    for k in range(NK):
                        nc.tensor.matmul(
                            out=ops[:],
                            lhsT=attT_sb[k][:, 128 * m : 128 * (m + 1)],
                            rhs=wo_sb[k][:, 512 * n : 512 * (n + 1)],
                            start=(k == 0),
                            stop=(k == NK - 1),
                        )
                    oev = outp.tile([128, 512], f32, tag="oev")
                    nc.vector.tensor_add(
                        out=oev, in0=ops[:], in1=bo_bc[:, 512 * n : 512 * (n + 1)]
                    )
                    nc.sync.dma_start(
                        out=out.ap()[128 * m : 128 * (m + 1), 512 * n : 512 * (n + 1)],
                        in_=oev,
                    )
    nc.finalize()
    return nc


def get_nc(use_f32r=True):
    key = use_f32r
    if key not in _COMPILED:
        _COMPILED[key] = build_nc(use_f32r)
    return _COMPILED[key]


def make_in_maps(x, Wq, bq, Wk, bk, Wv, bv, Wo, bo):
    bf = ml_dtypes.bfloat16
    x = np.asarray(x, np.float32)
    x_flat = x.reshape(B * S, D)
    weights = {
        "Wq": np.ascontiguousarray(np.asarray(Wq, np.float32).astype(bf)),
        "Wk": np.ascontiguousarray(np.asarray(Wk, np.float32).astype(bf)),
        "Wv": np.ascontiguousarray(np.asarray(Wv, np.float32).astype(bf)),
        "Wo": np.ascontiguousarray(np.asarray(Wo, np.float32).astype(bf)),
        "bq": np.asarray(bq, np.float32).reshape(D, 1),
        "bk": np.asarray(bk, np.float32).reshape(D, 1),
        "bv": np.asarray(bv, np.float32).reshape(D, 1),
        "bo": np.asarray(bo, np.float32).reshape(D, 1),
    }
    x_bf = x_flat.astype(bf)
    in_maps = []
    cpb = NCORES // B  # cores per batch
    for c in range(NCORES):
        b = c // cpb
        xb = x_bf[S * b : S * (b + 1), :]  # [2048, 1024] this core's batch
        # permute rows so this core's own 512 rows come first
        j = c % cpb
        perm = np.concatenate(
            [xb[512 * j : 512 * (j + 1)]]
            + [xb[512 * i : 512 * (i + 1)] for i in range(cpb) if i != j]
        )
        in_maps.append({"xTb": np.ascontiguousarray(perm.T), **weights})
    return in_maps


def gather_out(results):
    outs = [results[c]["out"] for c in range(NCORES)]
    return np.concatenate(outs, axis=0).reshape(B, S, D)


def kernel(x, Wq, bq, Wk, bk, Wv, bv, Wo, bo, _use_f32r=True):
    in_maps = make_in_maps(x, Wq, bq, Wk, bk, Wv, bv, Wo, bo)
    nc = get_nc(_use_f32r)
    res = run_bass_kernel_spmd(nc, in_maps, list(range(NCORES)))
    return gather_out(res.results)
